# revision 2
# baseline (speedup 1.0000x reference)
# Dynamic convolution (CondConv-style) Trainium2 Bass kernel.
#
# Problem: x [16, 128, 128, 128]; per-sample attention over K=4 expert
# 3x3 conv kernels; per-sample aggregated conv + bias.
#
# Strategy: data-parallel over batch, 2 samples per core on 8 cores.
# Per core, a software pipeline keeps the PE busy end-to-end:
#   - One-time: PE-transpose the K expert banks to [ci, tap, co]; zero the
#     halo borders of the two persistent padded-x slots.
#   - While sample s's conv runs (32 chunks x 9 PSUM-accumulated fp32r
#     matmuls, N=512), sample s+1's phase B rides along: its 16 x-load
#     DMAs + re-lay ops (fp32r rounding + pooled-mean accumulation) are
#     emitted between conv chunks, its 3 tiny attention matmuls are spread
#     between later chunks (so the in-order PE stream never stalls on
#     them), and its DVE expert mixing finishes before conv s ends.
#   - Drains (bias-add) on the scalar engine; DMA out per chunk.
import os

import numpy as np

B, C, H, W = 16, 128, 128, 128
K, HID, KS = 4, 64, 3
TEMP = 30.0
N_CORES = 8
BPC = B // N_CORES  # samples per core
HP, WP = H + 2, W + 2  # padded spatial
ROWS_PER_CHUNK = 4
NCHUNK = H // ROWS_PER_CHUNK
TAPS = KS * KS

_cache = {}


def _build(conv_f32r: bool, repeat: int = 1):
    """Build + compile the Bass program (same program for all 8 cores)."""
    from contextlib import ExitStack

    import concourse.bacc as bacc
    import concourse.mybir as mybir
    import concourse.tile as tile
    from concourse.bass import _add_dep_helper
    from concourse.masks import make_identity

    fp32 = mybir.dt.float32
    f32r = mybir.dt.float32r
    AF = mybir.ActivationFunctionType
    AX = mybir.AxisListType

    nc = bacc.Bacc(
        "TRN2",
        target_bir_lowering=False,
        debug=False,
        enable_asserts=False,
        num_devices=N_CORES,
    )

    x_d = nc.dram_tensor("x", (BPC, C, H, W), fp32, kind="ExternalInput").ap()
    w1_d = nc.dram_tensor("att_w1", (HID, C), fp32, kind="ExternalInput").ap()
    w2_d = nc.dram_tensor("att_w2", (K, HID), fp32, kind="ExternalInput").ap()
    wgt_d = nc.dram_tensor("weight", (K, C, C, KS, KS), fp32, kind="ExternalInput").ap()
    bias_d = nc.dram_tensor("bias", (K, C), fp32, kind="ExternalInput").ap()
    out_d = nc.dram_tensor("out", (BPC, C, H, W), fp32, kind="ExternalOutput").ap()

    wgt_flat = wgt_d.rearrange("k o i kh kw -> k o (i kh kw)")
    out_flat = out_d.rearrange("b c h w -> b c (h w)")
    conv_dt = f32r if conv_f32r else fp32

    QROWS = int(os.environ.get("KERNEL_QROWS", "8"))  # x staging chunk height
    nq = H // QROWS

    with tile.TileContext(nc) as tc, ExitStack() as ctx:
        consts = ctx.enter_context(tc.tile_pool(name="consts", bufs=1))
        xpool = ctx.enter_context(tc.tile_pool(name="xpool", bufs=1))
        smalls = ctx.enter_context(tc.tile_pool(name="smalls", bufs=1))
        stage = ctx.enter_context(tc.tile_pool(name="stage", bufs=int(os.environ.get("KERNEL_STAGE", "6"))))
        xtmp_pool = ctx.enter_context(tc.tile_pool(name="xtmp", bufs=int(os.environ.get("KERNEL_XTMP", "4"))))
        n_cps = int(os.environ.get("KERNEL_CPSUM", "7"))
        cpsum = ctx.enter_context(tc.tile_pool(name="cpsum", bufs=n_cps, space="PSUM"))
        spsum = ctx.enter_context(tc.tile_pool(name="spsum", bufs=1, space="PSUM"))

        # ---- global constants ----
        ident = consts.tile([C, C], fp32, name="ident")
        make_identity(nc, ident)
        ones_row = consts.tile([1, C], fp32, name="ones_row")
        nc.vector.memset(ones_row, 1.0)
        zero_col = consts.tile([C, HP], fp32, name="zero_col")
        nc.vector.memset(zero_col, 0.0)

        # attention weights: contiguous DMA + PE transpose (a strided DMA
        # of 4-byte elements would be far slower). w1T also absorbs the
        # 1/(H*W) mean normalization.
        w1_sb = consts.tile([HID, C], fp32, name="w1_sb")
        nc.sync.dma_start(out=w1_sb, in_=w1_d)
        w1T_ps = spsum.tile([C, HID], fp32, name="w1T_ps", tag="sps")
        nc.tensor.transpose(w1T_ps, w1_sb, ident[:HID, :HID])
        w1T = consts.tile([C, HID], fp32, name="w1T")
        nc.scalar.mul(w1T, w1T_ps, 1.0 / (H * W))

        w2_sb = consts.tile([K, HID], fp32, name="w2_sb")
        nc.sync.dma_start(out=w2_sb, in_=w2_d)
        w2T_ps = spsum.tile([HID, K], fp32, name="w2T_ps", tag="sps")
        nc.tensor.transpose(w2T_ps, w2_sb, ident[:K, :K])
        w2T = consts.tile([HID, K], fp32, name="w2T")
        nc.scalar.copy(w2T, w2T_ps)

        bias_sb = consts.tile([K, C], fp32, name="bias_sb")
        nc.sync.dma_start(out=bias_sb, in_=bias_d)
        biasT_ps = spsum.tile([C, K], fp32, name="biasT_ps", tag="sps")
        nc.tensor.transpose(biasT_ps, bias_sb, ident[:K, :K])
        biasT = consts.tile([C, K], fp32, name="biasT")
        nc.scalar.copy(biasT, biasT_ps)

        # ---- one-time: transpose expert banks to [ci, tap, co] ----
        wTe_pool = ctx.enter_context(tc.tile_pool(name="wTe", bufs=1))
        wTe = [wTe_pool.tile([C, TAPS, C], fp32, name=f"wTe{k}") for k in range(K)]
        wbank_dmas = []
        with tc.tile_pool(name="wbank", bufs=1) as wbank:
            w_sb = []
            for k in range(K):
                wk = wbank.tile([C, C, TAPS], fp32, name=f"w_sb{k}")
                wd = nc.sync.dma_start(
                    out=wk, in_=wgt_flat[k].rearrange("o (i t) -> o i t", t=TAPS)
                )
                wbank_dmas.append(wd)
                w_sb.append(wk)
            for k in range(K):
                for s in range(TAPS):
                    tp = cpsum.tile([C, C], fp32, name="tp", tag="cps")
                    nc.tensor.transpose(tp, w_sb[k][:, :, s], ident)
                    nc.vector.tensor_copy(wTe[k][:, s, :], tp)
        wTe_flat = [t.rearrange("c s o -> c (s o)") for t in wTe]

        # mix pool reuses the released wbank region (stack allocator)
        mix_pool = ctx.enter_context(tc.tile_pool(name="mix", bufs=1))
        wT_all = [
            mix_pool.tile([C, TAPS, C], conv_dt, name=f"wT{b}") for b in range(BPC)
        ]
        acc_t = mix_pool.tile([C, TAPS * C], fp32, name="acc_t")
        tmp_t = mix_pool.tile([C, TAPS * C], fp32, name="tmp_t")

        # ---- persistent per-slot tiles (slot = sample parity = b) ----
        x_pad = []
        for p in range(BPC):
            xp = xpool.tile([C, HP, WP], conv_dt, name=f"x_pad{p}", tag=f"xp{p}")
            # halo borders stay zero for the whole program
            nc.vector.tensor_copy(xp[:, 0, :], zero_col)
            nc.vector.tensor_copy(xp[:, HP - 1, :], zero_col)
            nc.vector.tensor_copy(xp[:, :, 0], zero_col)
            nc.vector.tensor_copy(xp[:, :, WP - 1], zero_col)
            x_pad.append(xp)

        def sm(shape, nm):
            return smalls.tile(shape, fp32, name=nm, tag=nm)

        partials = [sm([C, nq], f"partials{p}") for p in range(BPC)]
        pooled = [sm([C, 1], f"pooled{p}") for p in range(BPC)]
        h_sb = [sm([HID, 1], f"h_sb{p}") for p in range(BPC)]
        att_e = [sm([1, K], f"att_e{p}") for p in range(BPC)]
        esum = [sm([1, 1], f"esum{p}") for p in range(BPC)]
        rsum = [sm([1, 1], f"rsum{p}") for p in range(BPC)]
        att_row = [sm([1, K], f"att_row{p}") for p in range(BPC)]
        att_bc = [sm([C, K], f"att_bc{p}") for p in range(BPC)]
        btmp = [sm([C, K], f"btmp{p}") for p in range(BPC)]
        aggb = [sm([C, 1], f"aggb{p}") for p in range(BPC)]

        NS = repeat * BPC  # pipelined sample instances

        def emit_load(si, q, defer_wbank=False):
            """x-load chunk q of sample si + re-lay into the padded slot.

            The re-lay op both converts to fp32r (rounding the matmul
            verifier requires) and accumulates the chunk sum for the
            pooled mean. Alternate engines by parity so neither ACT nor
            DVE becomes the bottleneck."""
            b = si % BPC
            xt = xtmp_pool.tile([C, QROWS, W], fp32, name="xt")
            ld = nc.sync.dma_start(
                out=xt, in_=x_d[b, :, q * QROWS : (q + 1) * QROWS, :]
            )
            if defer_wbank and q == nq // 2:
                # weight bank rides behind the first half of x0
                for wd in wbank_dmas:
                    _add_dep_helper(wd.ins, ld.ins, reason="defer wbank")
            dst = x_pad[b][:, 1 + q * QROWS : 1 + (q + 1) * QROWS, 1 : W + 1]
            if b == 0:
                nc.scalar.activation(
                    dst, xt, AF.Copy, accum_out=partials[b][:, q : q + 1]
                )
            else:
                nc.vector.tensor_scalar(
                    dst,
                    xt,
                    1.0,
                    None,
                    op0=mybir.AluOpType.mult,
                    op1=mybir.AluOpType.add,
                    accum_out=partials[b][:, q : q + 1],
                )

        def emit_att1(si):
            b = si % BPC
            nc.vector.reduce_sum(out=pooled[b], in_=partials[b], axis=AX.X)
            h_ps = spsum.tile([HID, 1], fp32, name=f"h_ps{b}", tag="sps")
            nc.tensor.matmul(h_ps, w1T, pooled[b], start=True, stop=True)
            nc.scalar.activation(h_sb[b], h_ps, AF.Relu)

        def emit_att2(si):
            b = si % BPC
            log_ps = spsum.tile([1, K], fp32, name=f"log_ps{b}", tag="sps")
            nc.tensor.matmul(log_ps, h_sb[b], w2T, start=True, stop=True)
            # softmax: logits/TEMP are tiny (pooled means of unit
            # gaussians), so skip the max-subtraction; exp + sum in one op
            nc.scalar.activation(
                att_e[b], log_ps, AF.Exp, scale=1.0 / TEMP, accum_out=esum[b]
            )
            nc.vector.reciprocal(rsum[b], esum[b])
            nc.vector.tensor_scalar_mul(att_row[b], att_e[b], rsum[b])

        def emit_att3(si):
            b = si % BPC
            # broadcast normalized att to all partitions via K=1 matmul
            attb_ps = spsum.tile([C, K], fp32, name=f"attb_ps{b}", tag="sps")
            nc.tensor.matmul(attb_ps, ones_row, att_row[b], start=True, stop=True)
            nc.vector.tensor_copy(att_bc[b], attb_ps)
            # aggregated bias [C, 1]
            nc.vector.tensor_mul(btmp[b], biasT, att_bc[b])
            nc.vector.reduce_sum(out=aggb[b], in_=btmp[b], axis=AX.X)
            # expert mixing on DVE -> wT[b]
            wT_f = wT_all[b].rearrange("c s o -> c (s o)")
            nc.vector.tensor_scalar_mul(acc_t, wTe_flat[0], att_bc[b][:, 0:1])
            for k in (1, 2):
                nc.vector.tensor_scalar_mul(
                    tmp_t, wTe_flat[k], att_bc[b][:, k : k + 1]
                )
                nc.vector.tensor_add(acc_t, acc_t, tmp_t)
            nc.vector.tensor_scalar_mul(tmp_t, wTe_flat[3], att_bc[b][:, 3:4])
            nc.vector.tensor_add(wT_f, acc_t, tmp_t)

        def emit_conv_chunk(si, chunk, split_last):
            b = si % BPC
            h0 = chunk * ROWS_PER_CHUNK
            cps = cpsum.tile([C, ROWS_PER_CHUNK * W], fp32, name="cps", tag="cps")
            for s in range(TAPS):
                dy, dx = s // KS, s % KS
                rhs = x_pad[b][
                    :, h0 + dy : h0 + dy + ROWS_PER_CHUNK, dx : dx + W
                ]
                nc.tensor.matmul(
                    cps,
                    wT_all[b][:, s, :],
                    rhs,
                    start=(s == 0),
                    stop=(s == TAPS - 1),
                )
            if split_last:
                # split the very last drain so its DMA starts earlier
                half = ROWS_PER_CHUNK * W // 2
                for hh in range(2):
                    og2 = stage.tile([C, half], fp32, name="og2")
                    nc.scalar.activation(
                        og2,
                        cps[:, hh * half : (hh + 1) * half],
                        AF.Identity,
                        bias=aggb[b],
                        scale=1.0,
                    )
                    nc.sync.dma_start(
                        out=out_flat[
                            b, :, h0 * W + hh * half : h0 * W + (hh + 1) * half
                        ],
                        in_=og2,
                    )
            else:
                og = stage.tile([C, ROWS_PER_CHUNK * W], fp32, name="og")
                nc.scalar.activation(
                    og, cps, AF.Identity, bias=aggb[b], scale=1.0
                )
                nc.sync.dma_start(
                    out=out_flat[b, :, h0 * W : (h0 + ROWS_PER_CHUNK) * W],
                    in_=og,
                )

        # ---- startup: phase B of sample 0, unpipelined ----
        for q in range(nq):
            emit_load(0, q, defer_wbank=True)
        emit_att1(0)
        emit_att2(0)
        emit_att3(0)

        # ---- steady state: conv(si) with phase B of si+1 interleaved ----
        A1, A2, A3 = nq + 2, nq + 4, nq + 6
        for si in range(NS):
            has_next = si + 1 < NS
            for chunk in range(NCHUNK):
                emit_conv_chunk(si, chunk, split_last=(not has_next and chunk == NCHUNK - 1))
                if has_next:
                    if chunk < nq:
                        emit_load(si + 1, chunk)
                    elif chunk == A1:
                        emit_att1(si + 1)
                    elif chunk == A2:
                        emit_att2(si + 1)
                    elif chunk == A3:
                        emit_att3(si + 1)

    nc.compile()
    return nc


def _get_prog():
    conv_f32r = os.environ.get("KERNEL_CONV_DTYPE", "f32r") == "f32r"
    repeat = int(os.environ.get("KERNEL_REPEAT", "1"))
    key = (conv_f32r, repeat)
    if key not in _cache:
        _cache[key] = _build(conv_f32r, repeat)
    return _cache[key]


def kernel(x, att_w1, att_w2, weight, bias):
    from concourse.bass_utils import run_bass_kernel_spmd

    nc = _get_prog()
    in_maps = []
    for i in range(N_CORES):
        in_maps.append(
            {
                "x": np.ascontiguousarray(x[i * BPC : (i + 1) * BPC]),
                "att_w1": np.asarray(att_w1),
                "att_w2": np.asarray(att_w2),
                "weight": np.asarray(weight),
                "bias": np.asarray(bias),
            }
        )
    res = run_bass_kernel_spmd(nc, in_maps, list(range(N_CORES)))
    kernel.last_results = res
    return np.concatenate([r["out"] for r in res.results], axis=0)


# revision 5
# speedup vs baseline: 1.5838x; 1.5838x over previous
# Dynamic convolution (CondConv-style) Trainium2 Bass kernel.
#
# Problem: x [16, 128, 128, 128]; per-sample attention over K=4 expert
# 3x3 conv kernels; per-sample aggregated conv + bias.
#
# Strategy: data-parallel over batch, 2 samples per core on 8 cores.
# Per core, a software pipeline keeps the PE busy end-to-end:
#   - One-time: PE-transpose the K expert banks to [ci, tap, co]; zero the
#     halo borders of the two persistent padded-x slots.
#   - While sample s's conv runs (32 chunks x 9 PSUM-accumulated fp32r
#     matmuls, N=512), sample s+1's phase B rides along: its 16 x-load
#     DMAs + re-lay ops (fp32r rounding + pooled-mean accumulation) are
#     emitted between conv chunks, its 3 tiny attention matmuls are spread
#     between later chunks (so the in-order PE stream never stalls on
#     them), and its DVE expert mixing finishes before conv s ends.
#   - Drains (bias-add) on the scalar engine; DMA out per chunk.
import os

import numpy as np

B, C, H, W = 16, 128, 128, 128
K, HID, KS = 4, 64, 3
TEMP = 30.0
N_CORES = 8
BPC = B // N_CORES  # samples per core
HP, WP = H + 2, W + 2  # padded spatial
ROWS_PER_CHUNK = 4
NCHUNK = H // ROWS_PER_CHUNK
TAPS = KS * KS

_cache = {}


def _build(conv_f32r: bool, repeat: int = 1):
    """Build + compile the Bass program (same program for all 8 cores)."""
    from contextlib import ExitStack

    import concourse.bacc as bacc
    import concourse.mybir as mybir
    import concourse.tile as tile
    from concourse.bass import _add_dep_helper
    from concourse.masks import make_identity

    fp32 = mybir.dt.float32
    f32r = mybir.dt.float32r
    AF = mybir.ActivationFunctionType
    AX = mybir.AxisListType

    nc = bacc.Bacc(
        "TRN2",
        target_bir_lowering=False,
        debug=False,
        enable_asserts=False,
        num_devices=N_CORES,
    )

    x_d = nc.dram_tensor("x", (BPC, C, H, W), fp32, kind="ExternalInput").ap()
    w1_d = nc.dram_tensor("att_w1", (HID, C), fp32, kind="ExternalInput").ap()
    w2_d = nc.dram_tensor("att_w2", (K, HID), fp32, kind="ExternalInput").ap()
    wgt_d = nc.dram_tensor("weight", (K, C, C, KS, KS), fp32, kind="ExternalInput").ap()
    bias_d = nc.dram_tensor("bias", (K, C), fp32, kind="ExternalInput").ap()
    out_d = nc.dram_tensor("out", (BPC, C, H, W), fp32, kind="ExternalOutput").ap()

    wgt_flat = wgt_d.rearrange("k o i kh kw -> k o (i kh kw)")
    out_flat = out_d.rearrange("b c h w -> b c (h w)")
    conv_dt = f32r if conv_f32r else fp32

    QROWS = int(os.environ.get("KERNEL_QROWS", "8"))  # x staging chunk height
    nq = H // QROWS

    with tile.TileContext(nc) as tc, ExitStack() as ctx:
        consts = ctx.enter_context(tc.tile_pool(name="consts", bufs=1))
        xpool = ctx.enter_context(tc.tile_pool(name="xpool", bufs=1))
        smalls = ctx.enter_context(tc.tile_pool(name="smalls", bufs=1))
        stage = ctx.enter_context(tc.tile_pool(name="stage", bufs=int(os.environ.get("KERNEL_STAGE", "6"))))
        xtmp_pool = ctx.enter_context(tc.tile_pool(name="xtmp", bufs=int(os.environ.get("KERNEL_XTMP", "4"))))
        n_cps = int(os.environ.get("KERNEL_CPSUM", "7"))
        cpsum = ctx.enter_context(tc.tile_pool(name="cpsum", bufs=n_cps, space="PSUM"))
        spsum = ctx.enter_context(tc.tile_pool(name="spsum", bufs=1, space="PSUM"))

        # ---- global constants ----
        ident = consts.tile([C, C], fp32, name="ident")
        make_identity(nc, ident)
        ones_row = consts.tile([1, C], fp32, name="ones_row")
        nc.vector.memset(ones_row, 1.0)
        zero_col = consts.tile([C, HP], fp32, name="zero_col")
        nc.vector.memset(zero_col, 0.0)

        # attention weights: contiguous DMA + PE transpose (a strided DMA
        # of 4-byte elements would be far slower). w1T also absorbs the
        # 1/(H*W) mean normalization.
        w1_sb = consts.tile([HID, C], fp32, name="w1_sb")
        nc.sync.dma_start(out=w1_sb, in_=w1_d)
        w1T_ps = spsum.tile([C, HID], fp32, name="w1T_ps", tag="sps")
        nc.tensor.transpose(w1T_ps, w1_sb, ident[:HID, :HID])
        w1T = consts.tile([C, HID], fp32, name="w1T")
        nc.scalar.mul(w1T, w1T_ps, 1.0 / (H * W))

        w2_sb = consts.tile([K, HID], fp32, name="w2_sb")
        nc.sync.dma_start(out=w2_sb, in_=w2_d)
        w2T_ps = spsum.tile([HID, K], fp32, name="w2T_ps", tag="sps")
        nc.tensor.transpose(w2T_ps, w2_sb, ident[:K, :K])
        w2T = consts.tile([HID, K], fp32, name="w2T")
        nc.scalar.copy(w2T, w2T_ps)

        bias_sb = consts.tile([K, C], fp32, name="bias_sb")
        nc.sync.dma_start(out=bias_sb, in_=bias_d)
        biasT_ps = spsum.tile([C, K], fp32, name="biasT_ps", tag="sps")
        nc.tensor.transpose(biasT_ps, bias_sb, ident[:K, :K])
        biasT = consts.tile([C, K], fp32, name="biasT")
        nc.scalar.copy(biasT, biasT_ps)

        # ---- one-time: transpose expert banks to [ci, tap, co] ----
        wTe_pool = ctx.enter_context(tc.tile_pool(name="wTe", bufs=1))
        wTe = [wTe_pool.tile([C, TAPS, C], fp32, name=f"wTe{k}") for k in range(K)]
        wbank_dmas = []
        with tc.tile_pool(name="wbank", bufs=1) as wbank:
            w_sb = []
            for k in range(K):
                wk = wbank.tile([C, C, TAPS], fp32, name=f"w_sb{k}")
                wd = nc.sync.dma_start(
                    out=wk, in_=wgt_flat[k].rearrange("o (i t) -> o i t", t=TAPS)
                )
                wbank_dmas.append(wd)
                w_sb.append(wk)
            for k in range(K):
                for s in range(TAPS):
                    tp = cpsum.tile([C, C], fp32, name="tp", tag="cps")
                    nc.tensor.transpose(tp, w_sb[k][:, :, s], ident)
                    nc.vector.tensor_copy(wTe[k][:, s, :], tp)
        wTe_flat = [t.rearrange("c s o -> c (s o)") for t in wTe]

        # mix pool reuses the released wbank region (stack allocator)
        mix_pool = ctx.enter_context(tc.tile_pool(name="mix", bufs=1))
        wT_all = [
            mix_pool.tile([C, TAPS, C], conv_dt, name=f"wT{b}") for b in range(BPC)
        ]
        acc_t = mix_pool.tile([C, TAPS * C], fp32, name="acc_t")
        tmp_t = mix_pool.tile([C, TAPS * C], fp32, name="tmp_t")

        # ---- persistent per-slot tiles (slot = sample parity = b) ----
        x_pad = []
        for p in range(BPC):
            xp = xpool.tile([C, HP, WP], conv_dt, name=f"x_pad{p}", tag=f"xp{p}")
            # halo borders stay zero for the whole program
            nc.vector.tensor_copy(xp[:, 0, :], zero_col)
            nc.vector.tensor_copy(xp[:, HP - 1, :], zero_col)
            nc.vector.tensor_copy(xp[:, :, 0], zero_col)
            nc.vector.tensor_copy(xp[:, :, WP - 1], zero_col)
            x_pad.append(xp)

        def sm(shape, nm):
            return smalls.tile(shape, fp32, name=nm, tag=nm)

        partials = [sm([C, nq], f"partials{p}") for p in range(BPC)]
        pooled = [sm([C, 1], f"pooled{p}") for p in range(BPC)]
        h_sb = [sm([HID, 1], f"h_sb{p}") for p in range(BPC)]
        att_e = [sm([1, K], f"att_e{p}") for p in range(BPC)]
        esum = [sm([1, 1], f"esum{p}") for p in range(BPC)]
        rsum = [sm([1, 1], f"rsum{p}") for p in range(BPC)]
        att_row = [sm([1, K], f"att_row{p}") for p in range(BPC)]
        att_bc = [sm([C, K], f"att_bc{p}") for p in range(BPC)]
        btmp = [sm([C, K], f"btmp{p}") for p in range(BPC)]
        aggb = [sm([C, 1], f"aggb{p}") for p in range(BPC)]

        NS = repeat * BPC  # pipelined sample instances

        def emit_load(si, q, defer_wbank=False):
            """x-load chunk q of sample si + re-lay into the padded slot.

            The re-lay op both converts to fp32r (rounding the matmul
            verifier requires) and accumulates the chunk sum for the
            pooled mean. Alternate engines by parity so neither ACT nor
            DVE becomes the bottleneck."""
            b = si % BPC
            xt = xtmp_pool.tile([C, QROWS, W], fp32, name="xt")
            ld = nc.sync.dma_start(
                out=xt, in_=x_d[b, :, q * QROWS : (q + 1) * QROWS, :]
            )
            if defer_wbank and q == nq // 2:
                # weight bank rides behind the first half of x0
                for wd in wbank_dmas:
                    _add_dep_helper(wd.ins, ld.ins, reason="defer wbank")
            dst = x_pad[b][:, 1 + q * QROWS : 1 + (q + 1) * QROWS, 1 : W + 1]
            if b == 0:
                nc.scalar.activation(
                    dst, xt, AF.Copy, accum_out=partials[b][:, q : q + 1]
                )
            else:
                nc.vector.tensor_scalar(
                    dst,
                    xt,
                    1.0,
                    None,
                    op0=mybir.AluOpType.mult,
                    op1=mybir.AluOpType.add,
                    accum_out=partials[b][:, q : q + 1],
                )

        def emit_att1(si):
            b = si % BPC
            nc.vector.reduce_sum(out=pooled[b], in_=partials[b], axis=AX.X)
            h_ps = spsum.tile([HID, 1], fp32, name=f"h_ps{b}", tag="sps")
            nc.tensor.matmul(h_ps, w1T, pooled[b], start=True, stop=True)
            nc.scalar.activation(h_sb[b], h_ps, AF.Relu)

        def emit_att2(si):
            b = si % BPC
            log_ps = spsum.tile([1, K], fp32, name=f"log_ps{b}", tag="sps")
            nc.tensor.matmul(log_ps, h_sb[b], w2T, start=True, stop=True)
            # softmax: logits/TEMP are tiny (pooled means of unit
            # gaussians), so skip the max-subtraction; exp + sum in one op
            nc.scalar.activation(
                att_e[b], log_ps, AF.Exp, scale=1.0 / TEMP, accum_out=esum[b]
            )
            nc.vector.reciprocal(rsum[b], esum[b])
            nc.vector.tensor_scalar_mul(att_row[b], att_e[b], rsum[b])

        def emit_att3(si):
            b = si % BPC
            # broadcast normalized att to all partitions via K=1 matmul
            attb_ps = spsum.tile([C, K], fp32, name=f"attb_ps{b}", tag="sps")
            nc.tensor.matmul(attb_ps, ones_row, att_row[b], start=True, stop=True)
            nc.vector.tensor_copy(att_bc[b], attb_ps)
            # aggregated bias [C, 1]
            nc.vector.tensor_mul(btmp[b], biasT, att_bc[b])
            nc.vector.reduce_sum(out=aggb[b], in_=btmp[b], axis=AX.X)
            # expert mixing on DVE -> wT[b]
            wT_f = wT_all[b].rearrange("c s o -> c (s o)")
            nc.vector.tensor_scalar_mul(acc_t, wTe_flat[0], att_bc[b][:, 0:1])
            for k in (1, 2):
                nc.vector.tensor_scalar_mul(
                    tmp_t, wTe_flat[k], att_bc[b][:, k : k + 1]
                )
                nc.vector.tensor_add(acc_t, acc_t, tmp_t)
            nc.vector.tensor_scalar_mul(tmp_t, wTe_flat[3], att_bc[b][:, 3:4])
            nc.vector.tensor_add(wT_f, acc_t, tmp_t)

        def emit_conv_chunk(si, chunk, split_last):
            b = si % BPC
            h0 = chunk * ROWS_PER_CHUNK
            cps = cpsum.tile([C, ROWS_PER_CHUNK * W], fp32, name="cps", tag="cps")
            for s in range(TAPS):
                dy, dx = s // KS, s % KS
                rhs = x_pad[b][
                    :, h0 + dy : h0 + dy + ROWS_PER_CHUNK, dx : dx + W
                ]
                nc.tensor.matmul(
                    cps,
                    wT_all[b][:, s, :],
                    rhs,
                    start=(s == 0),
                    stop=(s == TAPS - 1),
                )
            st_eng = nc.scalar if os.environ.get("KERNEL_STQ") == "act" else nc.sync
            if split_last:
                # split the very last drain so its DMA starts earlier
                half = ROWS_PER_CHUNK * W // 2
                for hh in range(2):
                    og2 = stage.tile([C, half], fp32, name="og2")
                    nc.scalar.activation(
                        og2,
                        cps[:, hh * half : (hh + 1) * half],
                        AF.Identity,
                        bias=aggb[b],
                        scale=1.0,
                    )
                    st_eng.dma_start(
                        out=out_flat[
                            b, :, h0 * W + hh * half : h0 * W + (hh + 1) * half
                        ],
                        in_=og2,
                    )
            else:
                og = stage.tile([C, ROWS_PER_CHUNK * W], fp32, name="og")
                nc.scalar.activation(
                    og, cps, AF.Identity, bias=aggb[b], scale=1.0
                )
                st_eng.dma_start(
                    out=out_flat[b, :, h0 * W : (h0 + ROWS_PER_CHUNK) * W],
                    in_=og,
                )

        # ---- startup: phase B of sample 0, unpipelined ----
        for q in range(nq):
            emit_load(0, q, defer_wbank=True)
        emit_att1(0)
        emit_att2(0)
        emit_att3(0)

        # ---- steady state: conv(si) with phase B of si+1 interleaved ----
        att_boundary = os.environ.get("KERNEL_ATT") == "boundary"
        loads_front = os.environ.get("KERNEL_LOADS") == "front"
        A1, A2, A3 = nq + 2, nq + 4, nq + 6
        for si in range(NS):
            has_next = si + 1 < NS
            if has_next and loads_front:
                for q in range(nq):
                    emit_load(si + 1, q)
            for chunk in range(NCHUNK):
                emit_conv_chunk(si, chunk, split_last=(not has_next and chunk == NCHUNK - 1))
                if has_next:
                    if chunk < nq and not loads_front:
                        emit_load(si + 1, chunk)
                    elif not att_boundary and chunk == A1:
                        emit_att1(si + 1)
                    elif not att_boundary and chunk == A2:
                        emit_att2(si + 1)
                    elif not att_boundary and chunk == A3:
                        emit_att3(si + 1)
            if has_next and att_boundary:
                emit_att1(si + 1)
                emit_att2(si + 1)
                emit_att3(si + 1)

    nc.compile()
    return nc


def _get_prog():
    conv_f32r = os.environ.get("KERNEL_CONV_DTYPE", "f32r") == "f32r"
    repeat = int(os.environ.get("KERNEL_REPEAT", "1"))
    key = (conv_f32r, repeat)
    if key not in _cache:
        _cache[key] = _build(conv_f32r, repeat)
    return _cache[key]


def kernel(x, att_w1, att_w2, weight, bias):
    from concourse.bass_utils import run_bass_kernel_spmd

    nc = _get_prog()
    in_maps = []
    for i in range(N_CORES):
        in_maps.append(
            {
                "x": np.ascontiguousarray(x[i * BPC : (i + 1) * BPC]),
                "att_w1": np.asarray(att_w1),
                "att_w2": np.asarray(att_w2),
                "weight": np.asarray(weight),
                "bias": np.asarray(bias),
            }
        )
    res = run_bass_kernel_spmd(nc, in_maps, list(range(N_CORES)))
    kernel.last_results = res
    return np.concatenate([r["out"] for r in res.results], axis=0)


# revision 9
# speedup vs baseline: 1.6638x; 1.0506x over previous
# Dynamic convolution (CondConv-style) Trainium2 Bass kernel.
#
# Problem: x [16, 128, 128, 128]; per-sample attention over K=4 expert
# 3x3 conv kernels; per-sample aggregated conv + bias.
#
# Strategy: data-parallel over batch, 2 samples per core on 8 cores.
# Per core, a software pipeline keeps the PE busy end-to-end:
#   - One-time: PE-transpose the K expert banks to [ci, tap, co]; zero the
#     halo borders of the two persistent padded-x slots.
#   - While sample s's conv runs (32 chunks x 9 PSUM-accumulated fp32r
#     matmuls, N=512), sample s+1's phase B rides along: its 16 x-load
#     DMAs + re-lay ops (fp32r rounding + pooled-mean accumulation) are
#     emitted between conv chunks, its 3 tiny attention matmuls are spread
#     between later chunks (so the in-order PE stream never stalls on
#     them), and its DVE expert mixing finishes before conv s ends.
#   - Drains (bias-add) on the scalar engine; DMA out per chunk.
import os

import numpy as np

B, C, H, W = 16, 128, 128, 128
K, HID, KS = 4, 64, 3
TEMP = 30.0
N_CORES = 8
BPC = B // N_CORES  # samples per core
HP, WP = H + 2, W + 2  # padded spatial
TAPS = KS * KS

_cache = {}


def _build(conv_f32r: bool, repeat: int = 1):
    """Build + compile the Bass program (same program for all 8 cores)."""
    from contextlib import ExitStack

    import concourse.bacc as bacc
    import concourse.mybir as mybir
    import concourse.tile as tile
    from concourse.bass import _add_dep_helper
    from concourse.masks import make_identity

    fp32 = mybir.dt.float32
    f32r = mybir.dt.float32r
    AF = mybir.ActivationFunctionType
    AX = mybir.AxisListType

    nc = bacc.Bacc(
        "TRN2",
        target_bir_lowering=False,
        debug=False,
        enable_asserts=False,
        num_devices=N_CORES,
    )

    x_d = nc.dram_tensor("x", (BPC, C, H, W), fp32, kind="ExternalInput").ap()
    w1_d = nc.dram_tensor("att_w1", (HID, C), fp32, kind="ExternalInput").ap()
    w2_d = nc.dram_tensor("att_w2", (K, HID), fp32, kind="ExternalInput").ap()
    wgt_d = nc.dram_tensor("weight", (K, C, C, KS, KS), fp32, kind="ExternalInput").ap()
    bias_d = nc.dram_tensor("bias", (K, C), fp32, kind="ExternalInput").ap()
    out_d = nc.dram_tensor("out", (BPC, C, H, W), fp32, kind="ExternalOutput").ap()

    wgt_flat = wgt_d.rearrange("k o i kh kw -> k o (i kh kw)")
    out_flat = out_d.rearrange("b c h w -> b c (h w)")
    conv_dt = f32r if conv_f32r else fp32

    QROWS = int(os.environ.get("KERNEL_QROWS", "8"))  # x staging chunk height
    nq = H // QROWS
    ROWS_PER_CHUNK = int(os.environ.get("KERNEL_ROWS", "4"))  # conv chunk height
    NCHUNK = H // ROWS_PER_CHUNK

    with tile.TileContext(nc) as tc, ExitStack() as ctx:
        consts = ctx.enter_context(tc.tile_pool(name="consts", bufs=1))
        xpool = ctx.enter_context(tc.tile_pool(name="xpool", bufs=1))
        smalls = ctx.enter_context(tc.tile_pool(name="smalls", bufs=1))
        stage = ctx.enter_context(tc.tile_pool(name="stage", bufs=int(os.environ.get("KERNEL_STAGE", "6"))))
        xtmp_pool = ctx.enter_context(tc.tile_pool(name="xtmp", bufs=int(os.environ.get("KERNEL_XTMP", "4"))))
        n_cps = int(os.environ.get("KERNEL_CPSUM", "7"))
        cpsum = ctx.enter_context(tc.tile_pool(name="cpsum", bufs=n_cps, space="PSUM"))
        spsum = ctx.enter_context(tc.tile_pool(name="spsum", bufs=1, space="PSUM"))

        # ---- global constants ----
        ident = consts.tile([C, C], fp32, name="ident")
        make_identity(nc, ident)
        ones_row = consts.tile([1, C], fp32, name="ones_row")
        nc.vector.memset(ones_row, 1.0)
        zero_col = consts.tile([C, HP], fp32, name="zero_col")
        nc.vector.memset(zero_col, 0.0)

        # attention weights: contiguous DMA + PE transpose (a strided DMA
        # of 4-byte elements would be far slower). w1T also absorbs the
        # 1/(H*W) mean normalization.
        w1_sb = consts.tile([HID, C], fp32, name="w1_sb")
        nc.sync.dma_start(out=w1_sb, in_=w1_d)
        w1T_ps = spsum.tile([C, HID], fp32, name="w1T_ps", tag="sps")
        nc.tensor.transpose(w1T_ps, w1_sb, ident[:HID, :HID])
        w1T = consts.tile([C, HID], fp32, name="w1T")
        nc.scalar.mul(w1T, w1T_ps, 1.0 / (H * W))

        w2_sb = consts.tile([K, HID], fp32, name="w2_sb")
        nc.sync.dma_start(out=w2_sb, in_=w2_d)
        w2T_ps = spsum.tile([HID, K], fp32, name="w2T_ps", tag="sps")
        nc.tensor.transpose(w2T_ps, w2_sb, ident[:K, :K])
        w2T = consts.tile([HID, K], fp32, name="w2T")
        nc.scalar.copy(w2T, w2T_ps)

        bias_sb = consts.tile([K, C], fp32, name="bias_sb")
        nc.sync.dma_start(out=bias_sb, in_=bias_d)
        biasT_ps = spsum.tile([C, K], fp32, name="biasT_ps", tag="sps")
        nc.tensor.transpose(biasT_ps, bias_sb, ident[:K, :K])
        biasT = consts.tile([C, K], fp32, name="biasT")
        nc.scalar.copy(biasT, biasT_ps)

        # ---- one-time: transpose expert banks to [ci, tap, co] ----
        wTe_pool = ctx.enter_context(tc.tile_pool(name="wTe", bufs=1))
        wTe = [wTe_pool.tile([C, TAPS, C], fp32, name=f"wTe{k}") for k in range(K)]
        wbank_dmas = []
        with tc.tile_pool(name="wbank", bufs=1) as wbank:
            w_sb = []
            for k in range(K):
                wk = wbank.tile([C, C, TAPS], fp32, name=f"w_sb{k}")
                wd = nc.sync.dma_start(
                    out=wk, in_=wgt_flat[k].rearrange("o (i t) -> o i t", t=TAPS)
                )
                wbank_dmas.append(wd)
                w_sb.append(wk)
            for k in range(K):
                for s in range(TAPS):
                    tp = cpsum.tile([C, C], fp32, name="tp", tag="cps")
                    nc.tensor.transpose(tp, w_sb[k][:, :, s], ident)
                    nc.vector.tensor_copy(wTe[k][:, s, :], tp)
        wTe_flat = [t.rearrange("c s o -> c (s o)") for t in wTe]

        # mix pool reuses the released wbank region (stack allocator)
        mix_pool = ctx.enter_context(tc.tile_pool(name="mix", bufs=1))
        wT_all = [
            mix_pool.tile([C, TAPS, C], conv_dt, name=f"wT{b}") for b in range(BPC)
        ]
        acc_t = mix_pool.tile([C, TAPS * C], fp32, name="acc_t")
        tmp_t = mix_pool.tile([C, TAPS * C], fp32, name="tmp_t")

        # ---- persistent per-slot tiles (slot = sample parity = b) ----
        x_pad = []
        for p in range(BPC):
            xp = xpool.tile([C, HP, WP], conv_dt, name=f"x_pad{p}", tag=f"xp{p}")
            # halo borders stay zero for the whole program
            nc.vector.tensor_copy(xp[:, 0, :], zero_col)
            nc.vector.tensor_copy(xp[:, HP - 1, :], zero_col)
            nc.vector.tensor_copy(xp[:, :, 0], zero_col)
            nc.vector.tensor_copy(xp[:, :, WP - 1], zero_col)
            x_pad.append(xp)

        def sm(shape, nm):
            return smalls.tile(shape, fp32, name=nm, tag=nm)

        partials = [sm([C, nq], f"partials{p}") for p in range(BPC)]
        pooled = [sm([C, 1], f"pooled{p}") for p in range(BPC)]
        h_sb = [sm([HID, 1], f"h_sb{p}") for p in range(BPC)]
        att_e = [sm([1, K], f"att_e{p}") for p in range(BPC)]
        esum = [sm([1, 1], f"esum{p}") for p in range(BPC)]
        rsum = [sm([1, 1], f"rsum{p}") for p in range(BPC)]
        att_row = [sm([1, K], f"att_row{p}") for p in range(BPC)]
        att_bc = [sm([C, K], f"att_bc{p}") for p in range(BPC)]
        btmp = [sm([C, K], f"btmp{p}") for p in range(BPC)]
        aggb = [sm([C, 1], f"aggb{p}") for p in range(BPC)]

        NS = repeat * BPC  # pipelined sample instances

        def emit_load(si, q, defer_wbank=False):
            """x-load chunk q of sample si + re-lay into the padded slot.

            The re-lay op both converts to fp32r (rounding the matmul
            verifier requires) and accumulates the chunk sum for the
            pooled mean. Alternate engines by parity so neither ACT nor
            DVE becomes the bottleneck."""
            b = si % BPC
            xt = xtmp_pool.tile([C, QROWS, W], fp32, name="xt")
            ld = nc.sync.dma_start(
                out=xt, in_=x_d[b, :, q * QROWS : (q + 1) * QROWS, :]
            )
            if defer_wbank and q == nq // 2:
                # weight bank rides behind the first half of x0
                for wd in wbank_dmas:
                    _add_dep_helper(wd.ins, ld.ins, reason="defer wbank")
            dst = x_pad[b][:, 1 + q * QROWS : 1 + (q + 1) * QROWS, 1 : W + 1]
            if b == 0:
                nc.scalar.activation(
                    dst, xt, AF.Copy, accum_out=partials[b][:, q : q + 1]
                )
            else:
                nc.vector.tensor_scalar(
                    dst,
                    xt,
                    1.0,
                    None,
                    op0=mybir.AluOpType.mult,
                    op1=mybir.AluOpType.add,
                    accum_out=partials[b][:, q : q + 1],
                )

        def emit_att1(si):
            b = si % BPC
            nc.vector.reduce_sum(out=pooled[b], in_=partials[b], axis=AX.X)
            h_ps = spsum.tile([HID, 1], fp32, name=f"h_ps{b}", tag="sps")
            nc.tensor.matmul(h_ps, w1T, pooled[b], start=True, stop=True)
            nc.scalar.activation(h_sb[b], h_ps, AF.Relu)

        def emit_att2(si):
            b = si % BPC
            log_ps = spsum.tile([1, K], fp32, name=f"log_ps{b}", tag="sps")
            nc.tensor.matmul(log_ps, h_sb[b], w2T, start=True, stop=True)
            # softmax: logits/TEMP are tiny (pooled means of unit
            # gaussians), so skip the max-subtraction; exp + sum in one op
            nc.scalar.activation(
                att_e[b], log_ps, AF.Exp, scale=1.0 / TEMP, accum_out=esum[b]
            )
            nc.vector.reciprocal(rsum[b], esum[b])
            nc.vector.tensor_scalar_mul(att_row[b], att_e[b], rsum[b])

        def emit_att3(si):
            b = si % BPC
            # broadcast normalized att to all partitions via K=1 matmul
            attb_ps = spsum.tile([C, K], fp32, name=f"attb_ps{b}", tag="sps")
            nc.tensor.matmul(attb_ps, ones_row, att_row[b], start=True, stop=True)
            nc.vector.tensor_copy(att_bc[b], attb_ps)
            # aggregated bias [C, 1]
            nc.vector.tensor_mul(btmp[b], biasT, att_bc[b])
            nc.vector.reduce_sum(out=aggb[b], in_=btmp[b], axis=AX.X)
            # expert mixing on DVE -> wT[b]
            wT_f = wT_all[b].rearrange("c s o -> c (s o)")
            nc.vector.tensor_scalar_mul(acc_t, wTe_flat[0], att_bc[b][:, 0:1])
            for k in (1, 2):
                nc.vector.tensor_scalar_mul(
                    tmp_t, wTe_flat[k], att_bc[b][:, k : k + 1]
                )
                nc.vector.tensor_add(acc_t, acc_t, tmp_t)
            nc.vector.tensor_scalar_mul(tmp_t, wTe_flat[3], att_bc[b][:, 3:4])
            nc.vector.tensor_add(wT_f, acc_t, tmp_t)

        def emit_conv_chunk(si, chunk, split_last):
            b = si % BPC
            h0 = chunk * ROWS_PER_CHUNK
            cps = cpsum.tile([C, ROWS_PER_CHUNK * W], fp32, name="cps", tag="cps")
            for s in range(TAPS):
                dy, dx = s // KS, s % KS
                rhs = x_pad[b][
                    :, h0 + dy : h0 + dy + ROWS_PER_CHUNK, dx : dx + W
                ]
                nc.tensor.matmul(
                    cps,
                    wT_all[b][:, s, :],
                    rhs,
                    start=(s == 0),
                    stop=(s == TAPS - 1),
                )
            st_eng = nc.scalar if os.environ.get("KERNEL_STQ") == "act" else nc.sync
            if split_last:
                # split the very last drain so its DMA starts earlier
                half = ROWS_PER_CHUNK * W // 2
                for hh in range(2):
                    og2 = stage.tile([C, half], fp32, name="og2")
                    nc.scalar.activation(
                        og2,
                        cps[:, hh * half : (hh + 1) * half],
                        AF.Identity,
                        bias=aggb[b],
                        scale=1.0,
                    )
                    st_eng.dma_start(
                        out=out_flat[
                            b, :, h0 * W + hh * half : h0 * W + (hh + 1) * half
                        ],
                        in_=og2,
                    )
            else:
                og = stage.tile([C, ROWS_PER_CHUNK * W], fp32, name="og")
                nc.scalar.activation(
                    og, cps, AF.Identity, bias=aggb[b], scale=1.0
                )
                st_eng.dma_start(
                    out=out_flat[b, :, h0 * W : (h0 + ROWS_PER_CHUNK) * W],
                    in_=og,
                )

        # ---- startup: phase B of sample 0, unpipelined ----
        for q in range(nq):
            emit_load(0, q, defer_wbank=True)
        emit_att1(0)
        emit_att2(0)
        emit_att3(0)

        # ---- steady state: conv(si) with phase B of si+1 interleaved ----
        # Loads for si+1 are enqueued BEFORE conv(si)'s stores so the DMA
        # engine completes them early (interleaving ld/st in the queue
        # delays loads and stalls the PE at the attention matmuls).
        # The attention matmuls sit near the end of conv(si)'s in-order PE
        # stream; the DVE mixing they feed finishes just before conv(si+1).
        att_boundary = os.environ.get("KERNEL_ATT") == "boundary"
        loads_spread = os.environ.get("KERNEL_LOADS") == "spread"
        chunk_ns = TAPS * ROWS_PER_CHUNK * W * 0.4167
        A3 = NCHUNK - 1 - max(1, int(9600 / chunk_ns + 0.999))
        A1, A2 = A3 - 2, A3 - 1
        for si in range(NS):
            has_next = si + 1 < NS
            if has_next and not loads_spread:
                for q in range(nq):
                    emit_load(si + 1, q)
            for chunk in range(NCHUNK):
                emit_conv_chunk(si, chunk, split_last=(not has_next and chunk == NCHUNK - 1))
                if has_next:
                    if chunk < nq and loads_spread:
                        emit_load(si + 1, chunk)
                    elif not att_boundary and chunk == A1:
                        emit_att1(si + 1)
                    elif not att_boundary and chunk == A2:
                        emit_att2(si + 1)
                    elif not att_boundary and chunk == A3:
                        emit_att3(si + 1)
            if has_next and att_boundary:
                emit_att1(si + 1)
                emit_att2(si + 1)
                emit_att3(si + 1)

    nc.compile()
    return nc


def _get_prog():
    conv_f32r = os.environ.get("KERNEL_CONV_DTYPE", "f32r") == "f32r"
    repeat = int(os.environ.get("KERNEL_REPEAT", "1"))
    key = (conv_f32r, repeat)
    if key not in _cache:
        _cache[key] = _build(conv_f32r, repeat)
    return _cache[key]


def kernel(x, att_w1, att_w2, weight, bias):
    from concourse.bass_utils import run_bass_kernel_spmd

    nc = _get_prog()
    in_maps = []
    for i in range(N_CORES):
        in_maps.append(
            {
                "x": np.ascontiguousarray(x[i * BPC : (i + 1) * BPC]),
                "att_w1": np.asarray(att_w1),
                "att_w2": np.asarray(att_w2),
                "weight": np.asarray(weight),
                "bias": np.asarray(bias),
            }
        )
    res = run_bass_kernel_spmd(nc, in_maps, list(range(N_CORES)))
    kernel.last_results = res
    return np.concatenate([r["out"] for r in res.results], axis=0)
